# revision 5
# baseline (speedup 1.0000x reference)
"""Trainium2 Bass kernel for the nn_CA depthwise-conv CA step.

Pipeline per image: depthwise 5x5 conv (D4-symmetrized, zero-mean kernel,
SAME padding) + bias + leaky_relu; 1x1 conv (8x8 channel GEMM) + bias +
leaky_relu; 1x1 conv + bias + psi residual + tanh.

Strategy: pure data parallel over 8 NeuronCores (256 images each).
On-chip layout: partition p = h*16 + ylocal, one tile per y-quarter
(y = qy*16 + ylocal) holding ALL 8 channels; free dim = (image, x).
All data is fp16 (halves DMA/SBUF vs fp32; PE matmul rate is 1 row/cycle
either way).

With all 8 channels in the partition dim, each 8x8 channel GEMM is a
SINGLE matmul per quarter (lhsT = W[co,ci] (x) I16), vs 4 accumulating
matmuls per output pair in a (2ch x 64y) layout — stage 2+3 drop from 32
to 8 matmuls per 8-image step. The depthwise conv uses the D4 x-symmetry
(kernel columns 0==4, 1==3): per quarter it is 3 banded-Toeplitz matmuls
(center column, s1 = psi(x-1)+psi(x+1), s2 = psi(x-2)+psi(x+2), the s
tiles built on the idle vector engine), plus corner-block "spill"
matmuls that carry the y-convolution across quarter boundaries (up to 6
per quarter). Total 38 matmuls per step vs 44 for the pair layout. The
psi residual is added by the vector engine in PSUM; biases + activations
are fused into ScalarE activation instructions reading PSUM.
"""

import numpy as np

F16 = np.dtype(np.float16)

BS, H, RES = 2048, 8, 64
NCORES = 8
IPC = BS // NCORES  # images per core
SG = 32             # images per super-group (DMA granularity)
NSG = IPC // SG
S8 = 8              # images per PSUM step (512 free columns per quarter)
NST = SG // S8
XP = RES + 4        # x-padded width
YQ = 16             # y rows per quarter tile

NWM = 12            # 3 main conv + 3 spill-up + 3 spill-down + gemm2 + gemm3 + I

_CACHE = {}


def _totalistic(x):
    # D4-symmetrize 5x5 kernels over spatial dims, then remove spatial mean
    z = 0.125 * (x + x[:, :, ::-1, :] + x[:, :, :, ::-1] + x[:, :, ::-1, ::-1])
    xt = np.swapaxes(x, 2, 3)
    z = z + 0.125 * (xt + xt[:, :, ::-1, :] + xt[:, :, :, ::-1] + xt[:, :, ::-1, ::-1])
    return z - z.mean(axis=(2, 3), keepdims=True)


def _build_program(reps=1, resid_on_pe=False):
    import concourse.bacc as bacc
    import concourse.tile as tile
    from concourse import mybir

    dt = mybir.dt
    nc = bacc.Bacc("TRN2", target_bir_lowering=False, debug=False, num_devices=NCORES)
    psi = nc.dram_tensor("psi", [4, 128, IPC * XP], dt.float16, kind="ExternalInput").ap()
    wm = nc.dram_tensor("wm", [NWM, 128, 128], dt.float16, kind="ExternalInput").ap()
    bv = nc.dram_tensor("bv", [128, 2], dt.float32, kind="ExternalInput").ap()
    out = nc.dram_tensor("out", [4, 128, IPC * RES], dt.float16, kind="ExternalOutput").ap()

    LR = mybir.ActivationFunctionType.Lrelu
    TH = mybir.ActivationFunctionType.Tanh

    with tile.TileContext(nc) as tc:
        from contextlib import ExitStack

        with ExitStack() as ctx:
            const = ctx.enter_context(tc.tile_pool(name="const", bufs=1))
            psip = ctx.enter_context(tc.tile_pool(name="psip", bufs=2))
            spool = ctx.enter_context(tc.tile_pool(name="spool", bufs=2))
            opool = ctx.enter_context(tc.tile_pool(name="opool", bufs=2))
            zpool = ctx.enter_context(tc.tile_pool(name="zpool", bufs=2))
            psum = ctx.enter_context(tc.tile_pool(name="psum", bufs=4, space="PSUM"))

            wt = const.tile([128, NWM * 128], dt.float16)
            nc.sync.dma_start(
                wt[:].rearrange("p (m k) -> p m k", m=NWM),
                wm.rearrange("m p k -> p m k"),
            )
            bt = const.tile([128, 2], dt.float32)
            nc.sync.dma_start(bt[:], bv[:])

            def W(i):
                return wt[:, i * 128 : (i + 1) * 128]

            for rep in range(reps):
              for sg in range(NSG):
                ptiles, s1tiles, s2tiles = [], [], []
                for v in range(4):
                    t = psip.tile([128, SG * XP], dt.float16, tag=f"psi{v}", name=f"psi_t{v}_{sg}")
                    nc.sync.dma_start(
                        t[:], psi[v, :, sg * SG * XP : (sg + 1) * SG * XP]
                    )
                    ptiles.append(t)
                    pv = t[:].rearrange("p (i x) -> p i x", i=SG)
                    t1 = spool.tile([128, SG * RES], dt.float16, tag=f"s1_{v}", name=f"s1_t{v}_{sg}")
                    t2 = spool.tile([128, SG * RES], dt.float16, tag=f"s2_{v}", name=f"s2_t{v}_{sg}")
                    v1 = t1[:].rearrange("p (i x) -> p i x", i=SG)
                    v2 = t2[:].rearrange("p (i x) -> p i x", i=SG)
                    nc.vector.tensor_add(v1, pv[:, :, 1 : 1 + RES], pv[:, :, 3 : 3 + RES])
                    nc.vector.tensor_add(v2, pv[:, :, 0:RES], pv[:, :, 4 : 4 + RES])
                    s1tiles.append(t1)
                    s2tiles.append(t2)
                otiles = [
                    opool.tile([128, SG * RES], dt.float16, tag=f"o{v}", name=f"o_t{v}_{sg}")
                    for v in range(4)
                ]

                def views(v, st):
                    # (center, s1, s2) rhs views of quarter v for step st
                    pt3 = ptiles[v][:].rearrange("p (i x) -> p i x", i=SG)[
                        :, st * S8 : (st + 1) * S8, :
                    ]
                    s13 = s1tiles[v][:].rearrange("p (i x) -> p i x", i=SG)[
                        :, st * S8 : (st + 1) * S8, :
                    ]
                    s23 = s2tiles[v][:].rearrange("p (i x) -> p i x", i=SG)[
                        :, st * S8 : (st + 1) * S8, :
                    ]
                    return (pt3[:, :, 2 : 2 + RES], s13, s23)

                for st in range(NST):
                    # stage 1: depthwise conv + lrelu (bias1 is always 0).
                    # cps spans 4 PSUM banks; each quarter's accumulation
                    # group stays within its own 512-fp32 bank.
                    cps = psum.tile(
                        [128, 4 * S8 * RES], dt.float32,
                        tag="cps", name=f"cps{sg}_{st}", bufs=1,
                    )
                    for v in range(4):
                        cps3 = cps[:, v * S8 * RES : (v + 1) * S8 * RES].rearrange(
                            "p (i x) -> p i x", i=S8
                        )
                        mm = []
                        own = views(v, st)
                        for c in range(3):  # lhsT index c: x-class (0=s2,1=s1,2=center)
                            mm.append((W(c), own[2 - c]))
                        if v > 0:
                            below = views(v - 1, st)
                            for c in range(3):
                                mm.append((W(3 + c), below[2 - c]))
                        if v < 3:
                            above = views(v + 1, st)
                            for c in range(3):
                                mm.append((W(6 + c), above[2 - c]))
                        for j, (lhs, rhs) in enumerate(mm):
                            nc.tensor.matmul(
                                cps3, lhs, rhs,
                                start=(j == 0), stop=(j == len(mm) - 1),
                            )
                    z1 = zpool.tile(
                        [128, 4 * S8 * RES], dt.float16,
                        tag="z1", name=f"z1_{sg}_{st}",
                    )
                    # per-quarter acts: bank v drains while PE convolves v+1
                    for v in range(4):
                        sl = slice(v * S8 * RES, (v + 1) * S8 * RES)
                        nc.scalar.activation(z1[:, sl], cps[:, sl], LR, alpha=0.01)
                    # stage 2: 8x8 channel GEMM + b2 + lrelu (one matmul/quarter)
                    z2 = []
                    for v in range(4):
                        gps = psum.tile(
                            [128, S8 * RES], dt.float32, tag="ps", name=f"g2ps{sg}_{st}_{v}",
                        )
                        nc.tensor.matmul(
                            gps[:], W(9),
                            z1[:, v * S8 * RES : (v + 1) * S8 * RES],
                            start=True, stop=True,
                        )
                        z = zpool.tile([128, S8 * RES], dt.float16, tag=f"z2_{v}", name=f"z2_{sg}_{st}_{v}")
                        nc.scalar.activation(
                            z[:], gps[:], LR, bias=bt[:, 0:1], alpha=0.01
                        )
                        z2.append(z)
                    # stage 3: 8x8 channel GEMM + psi residual + b3 + tanh
                    for v in range(4):
                        gps = psum.tile(
                            [128, S8 * RES], dt.float32, tag="ps", name=f"g3ps{sg}_{st}_{v}",
                        )
                        pt3 = ptiles[v][:].rearrange("p (i x) -> p i x", i=SG)[
                            :, st * S8 : (st + 1) * S8, 2 : 2 + RES
                        ]
                        gps3 = gps[:].rearrange("p (i x) -> p i x", i=S8)
                        if resid_on_pe:
                            nc.tensor.matmul(gps3, W(11), pt3, start=True, stop=False)
                        nc.tensor.matmul(
                            gps[:], W(10), z2[v][:],
                            start=not resid_on_pe, stop=True,
                        )
                        if not resid_on_pe:
                            # psi residual on the vector engine
                            nc.vector.tensor_add(gps3, gps3, pt3)
                        nc.scalar.activation(
                            otiles[v][:, st * S8 * RES : (st + 1) * S8 * RES],
                            gps[:], TH, bias=bt[:, 1:2],
                        )
                for v in range(4):
                    nc.sync.dma_start(
                        out[v, :, sg * SG * RES : (sg + 1) * SG * RES],
                        otiles[v][:],
                    )

    nc.compile()
    return nc


def _host_pack(filter1, bias1, w2, b2, w3, b3):
    w = _totalistic(filter1.astype(np.float32))[:, 0]  # [8,5,5]
    wm = np.zeros((NWM, 128, 128), np.float32)
    # main banded conv matrices (same local structure for every quarter):
    # lhsT[ch*16+yi, ch*16+yo] = K[ch, (yi-yo)+2, c]
    for c in range(3):
        for ch in range(H):
            o = ch * YQ
            for yi in range(YQ):
                for yo in range(YQ):
                    d = yi - yo
                    if -2 <= d <= 2:
                        wm[c, o + yi, o + yo] = w[ch, d + 2, c]
        # spill up: src quarter u -> dst u+1; dy = ys - 16 - yd in {-2,-1}
        for ch in range(H):
            o = ch * YQ
            for ys, yd in ((14, 0), (15, 0), (15, 1)):
                wm[3 + c, o + ys, o + yd] = w[ch, (ys - 16 - yd) + 2, c]
        # spill down: src quarter u -> dst u-1; dy = ys + 16 - yd in {1,2}
        for ch in range(H):
            o = ch * YQ
            for ys, yd in ((0, 14), (0, 15), (1, 15)):
                wm[6 + c, o + ys, o + yd] = w[ch, (ys + 16 - yd) + 2, c]
    # channel GEMMs: lhsT[ci*16+y, co*16+y] = W[co, ci]
    i16 = np.eye(YQ, dtype=np.float32)
    for co in range(H):
        for ci in range(H):
            wm[9, ci * YQ : ci * YQ + YQ, co * YQ : co * YQ + YQ] = w2[co, ci] * i16
            wm[10, ci * YQ : ci * YQ + YQ, co * YQ : co * YQ + YQ] = w3[co, ci] * i16
    wm[11] = np.eye(128, dtype=np.float32)

    bvv = np.zeros((128, 2), np.float32)
    for ch in range(H):
        sl = slice(ch * YQ, ch * YQ + YQ)
        bvv[sl, 0] = b2[ch]
        bvv[sl, 1] = b3[ch]
    return wm.astype(F16), bvv


def _pack_psi(psi):
    """[BS,H,RES,RES] -> [NCORES, 4, 128, IPC*XP] fp16: tile axis = y-quarter,
    partition p = ch*16 + ylocal, free = (image, padded x)."""
    psip = np.zeros((BS, H, RES, XP), np.float32)
    psip[:, :, :, 2 : 2 + RES] = psi
    v = psip.reshape(NCORES, IPC, H, 4, YQ, XP)
    v = v.transpose(0, 3, 2, 4, 1, 5)  # [NCORES, qy, ch, ylocal, IPC, XP]
    return np.ascontiguousarray(v).reshape(NCORES, 4, 128, IPC * XP).astype(F16)


def _unpack_out(parts):
    """list of [4,128,IPC*RES] fp16 per core -> [BS,H,RES,RES] fp32"""
    v = np.stack([np.asarray(p).astype(np.float32) for p in parts])
    v = v.reshape(NCORES, 4, H, YQ, IPC, RES)
    v = v.transpose(0, 4, 2, 1, 3, 5)  # [NCORES, IPC, ch, qy, ylocal, RES]
    return np.ascontiguousarray(v).reshape(BS, H, RES, RES)


def kernel(psi, filter1, bias1, w2, b2, w3, b3):
    from concourse.bass_utils import run_bass_kernel_spmd

    psi = np.asarray(psi, dtype=np.float32)
    wm, bvv = _host_pack(
        np.asarray(filter1, np.float32),
        np.asarray(bias1, np.float32),
        np.asarray(w2, np.float32),
        np.asarray(b2, np.float32),
        np.asarray(w3, np.float32),
        np.asarray(b3, np.float32),
    )

    psit = _pack_psi(psi)

    if "nc" not in _CACHE:
        _CACHE["nc"] = _build_program()
    nc = _CACHE["nc"]

    in_maps = [{"psi": psit[c], "wm": wm, "bv": bvv} for c in range(NCORES)]
    res = run_bass_kernel_spmd(nc, in_maps, list(range(NCORES)))
    return _unpack_out([r["out"] for r in res.results])


# revision 6
# speedup vs baseline: 1.2856x; 1.2856x over previous
"""Trainium2 Bass kernel for the nn_CA depthwise-conv CA step.

Pipeline per image: depthwise 5x5 conv (D4-symmetrized, zero-mean kernel,
SAME padding) + bias + leaky_relu; 1x1 conv (8x8 channel GEMM) + bias +
leaky_relu; 1x1 conv + bias + psi residual + tanh.

Strategy: pure data parallel over 8 NeuronCores (256 images each).
On-chip layout: partition p = b*64 + y for a channel pair (2q, 2q+1);
free dim = (image, x). All data is fp16 (DMA traffic and SBUF halved vs
fp32; PE matmul rate is 1 row/cycle for both).

The depthwise conv exploits the D4 x-symmetry of the kernel columns
(K[:,:,0]==K[:,:,4], K[:,:,1]==K[:,:,3]): per channel pair it is only 3
matmuls — a banded-Toeplitz lhsT (y-convolution) applied to the center
column, to s1 = psi(x-1)+psi(x+1), and to s2 = psi(x-2)+psi(x+2). The
s1/s2 shifted sums are built on the (otherwise idle) vector engine from
the x-padded psi tile, so no extra HBM traffic. The 8x8 channel GEMMs
are 4 accumulating matmuls per output pair with lhsT = W[co,ci] x I64
blocks. The psi residual is added by the vector engine in PSUM; biases +
leaky_relu/tanh are fused into ScalarE activations reading PSUM.
"""

import numpy as np
F16 = np.dtype(np.float16)

BS, H, RES = 2048, 8, 64
NCORES = 8
IPC = BS // NCORES  # images per core
SG = 32             # images per super-group (DMA granularity)
NSG = IPC // SG
S8 = 8              # images per PSUM step (512 free columns)
NST = SG // S8
XP = RES + 4        # x-padded width

NWM = 45            # 12 conv + 16 gemm2 + 16 gemm3 + identity

_CACHE = {}


def _totalistic(x):
    # D4-symmetrize 5x5 kernels over spatial dims, then remove spatial mean
    z = 0.125 * (x + x[:, :, ::-1, :] + x[:, :, :, ::-1] + x[:, :, ::-1, ::-1])
    xt = np.swapaxes(x, 2, 3)
    z = z + 0.125 * (xt + xt[:, :, ::-1, :] + xt[:, :, :, ::-1] + xt[:, :, ::-1, ::-1])
    return z - z.mean(axis=(2, 3), keepdims=True)


def _build_program(reps=1, resid_on_pe=False):
    import concourse.bacc as bacc
    import concourse.tile as tile
    from concourse import mybir

    dt = mybir.dt
    nc = bacc.Bacc("TRN2", target_bir_lowering=False, debug=False, num_devices=NCORES)
    psi = nc.dram_tensor("psi", [4, 128, IPC * XP], dt.float16, kind="ExternalInput").ap()
    wm = nc.dram_tensor("wm", [NWM, 128, 128], dt.float16, kind="ExternalInput").ap()
    bv = nc.dram_tensor("bv", [128, 12], dt.float32, kind="ExternalInput").ap()
    out = nc.dram_tensor("out", [4, 128, IPC * RES], dt.float16, kind="ExternalOutput").ap()

    LR = mybir.ActivationFunctionType.Lrelu
    TH = mybir.ActivationFunctionType.Tanh

    with tile.TileContext(nc) as tc:
        from contextlib import ExitStack

        with ExitStack() as ctx:
            const = ctx.enter_context(tc.tile_pool(name="const", bufs=1))
            psip = ctx.enter_context(tc.tile_pool(name="psip", bufs=2))
            spool = ctx.enter_context(tc.tile_pool(name="spool", bufs=2))
            opool = ctx.enter_context(tc.tile_pool(name="opool", bufs=2))
            zpool = ctx.enter_context(tc.tile_pool(name="zpool", bufs=2))
            psum = ctx.enter_context(tc.tile_pool(name="psum", bufs=4, space="PSUM"))

            wt = const.tile([128, NWM * 128], dt.float16)
            nc.sync.dma_start(
                wt[:].rearrange("p (m k) -> p m k", m=NWM),
                wm.rearrange("m p k -> p m k"),
            )
            bt = const.tile([128, 12], dt.float32)
            nc.sync.dma_start(bt[:], bv[:])

            def W(i):
                return wt[:, i * 128 : (i + 1) * 128]

            for rep in range(reps):
              for sg in range(NSG):
                ptiles, s1tiles, s2tiles = [], [], []
                for q in range(4):
                    t = psip.tile([128, SG * XP], dt.float16, tag=f"psi{q}", name=f"psi_t{q}_{sg}")
                    nc.sync.dma_start(
                        t[:], psi[q, :, sg * SG * XP : (sg + 1) * SG * XP]
                    )
                    ptiles.append(t)
                    pv = t[:].rearrange("p (i x) -> p i x", i=SG)
                    t1 = spool.tile([128, SG * RES], dt.float16, tag=f"s1_{q}", name=f"s1_t{q}_{sg}")
                    t2 = spool.tile([128, SG * RES], dt.float16, tag=f"s2_{q}", name=f"s2_t{q}_{sg}")
                    v1 = t1[:].rearrange("p (i x) -> p i x", i=SG)
                    v2 = t2[:].rearrange("p (i x) -> p i x", i=SG)
                    nc.vector.tensor_add(v1, pv[:, :, 1 : 1 + RES], pv[:, :, 3 : 3 + RES])
                    nc.vector.tensor_add(v2, pv[:, :, 0:RES], pv[:, :, 4 : 4 + RES])
                    s1tiles.append(t1)
                    s2tiles.append(t2)
                otiles = [
                    opool.tile([128, SG * RES], dt.float16, tag=f"o{q}", name=f"o_t{q}_{sg}")
                    for q in range(4)
                ]
                for st in range(NST):
                    # stage 1: depthwise conv + lrelu (bias1 is always 0,
                    # so pairs share one merged activation over 2 PSUM banks)
                    z1big = []
                    for jj in range(2):
                        cps = psum.tile(
                            [128, 2 * S8 * RES], dt.float32,
                            tag="cps", name=f"cps{sg}_{st}_{jj}", bufs=2,
                        )
                        for h in range(2):
                            q = 2 * jj + h
                            cps3 = cps[:, h * S8 * RES : (h + 1) * S8 * RES].rearrange(
                                "p (i x) -> p i x", i=S8
                            )
                            pt3 = ptiles[q][:].rearrange("p (i x) -> p i x", i=SG)[
                                :, st * S8 : (st + 1) * S8, :
                            ]
                            s13 = s1tiles[q][:].rearrange("p (i x) -> p i x", i=SG)[
                                :, st * S8 : (st + 1) * S8, :
                            ]
                            s23 = s2tiles[q][:].rearrange("p (i x) -> p i x", i=SG)[
                                :, st * S8 : (st + 1) * S8, :
                            ]
                            nc.tensor.matmul(
                                cps3, W(q * 3 + 2), pt3[:, :, 2 : 2 + RES],
                                start=True, stop=False,
                            )
                            nc.tensor.matmul(cps3, W(q * 3 + 1), s13, start=False, stop=False)
                            nc.tensor.matmul(cps3, W(q * 3), s23, start=False, stop=True)
                        z = zpool.tile(
                            [128, 2 * S8 * RES], dt.float16,
                            tag=f"z1_{jj}", name=f"z1_{sg}_{st}_{jj}",
                        )
                        nc.scalar.activation(z[:], cps[:], LR, alpha=0.01)
                        z1big.append(z)
                    z1 = [
                        z1big[q // 2][:, (q % 2) * S8 * RES : (q % 2 + 1) * S8 * RES]
                        for q in range(4)
                    ]
                    # stage 2: 8x8 channel GEMM + b2 + lrelu
                    z2 = []
                    for p in range(4):
                        gps = psum.tile(
                            [128, S8 * RES], dt.float32, tag="ps", name=f"g2ps{sg}_{st}_{p}",
                        )
                        for q in range(4):
                            nc.tensor.matmul(
                                gps[:], W(12 + p * 4 + q), z1[q],
                                start=(q == 0), stop=(q == 3),
                            )
                        z = zpool.tile([128, S8 * RES], dt.float16, tag=f"z2_{p}", name=f"z2_{sg}_{st}_{p}")
                        nc.scalar.activation(
                            z[:], gps[:], LR, bias=bt[:, 4 + p : 5 + p], alpha=0.01
                        )
                        z2.append(z)
                    # stage 3: 8x8 channel GEMM + psi residual + b3 + tanh
                    for p in range(4):
                        gps = psum.tile(
                            [128, S8 * RES], dt.float32, tag="ps", name=f"g3ps{sg}_{st}_{p}",
                        )
                        pt3 = ptiles[p][:].rearrange("p (i x) -> p i x", i=SG)[
                            :, st * S8 : (st + 1) * S8, 2 : 2 + RES
                        ]
                        gps3 = gps[:].rearrange("p (i x) -> p i x", i=S8)
                        if resid_on_pe:
                            nc.tensor.matmul(gps3, W(44), pt3, start=True, stop=False)
                        for q in range(4):
                            nc.tensor.matmul(
                                gps[:], W(28 + p * 4 + q), z2[q][:],
                                start=(q == 0) and not resid_on_pe, stop=(q == 3),
                            )
                        if not resid_on_pe:
                            # psi residual on the vector engine
                            nc.vector.tensor_add(gps3, gps3, pt3)
                        nc.scalar.activation(
                            otiles[p][:, st * S8 * RES : (st + 1) * S8 * RES],
                            gps[:], TH, bias=bt[:, 8 + p : 9 + p],
                        )
                for q in range(4):
                    nc.sync.dma_start(
                        out[q, :, sg * SG * RES : (sg + 1) * SG * RES],
                        otiles[q][:],
                    )

    nc.compile()
    return nc


def _host_pack(filter1, bias1, w2, b2, w3, b3):
    w = _totalistic(filter1.astype(np.float32))[:, 0]  # [8,5,5]
    wm = np.zeros((NWM, 128, 128), np.float32)
    eye = {d: np.eye(RES, k=-d, dtype=np.float32) for d in range(-2, 3)}
    for q in range(4):
        for c in range(3):  # kernel x-class c (columns c and 4-c are equal)
            m = wm[q * 3 + c]
            for b in range(2):
                blk = np.zeros((RES, RES), np.float32)
                for d in range(-2, 3):
                    blk += w[2 * q + b, d + 2, c] * eye[d]
                m[b * 64 : b * 64 + 64, b * 64 : b * 64 + 64] = blk
    i64 = np.eye(64, dtype=np.float32)
    for p in range(4):
        for q in range(4):
            m2 = wm[12 + p * 4 + q]
            m3 = wm[28 + p * 4 + q]
            for bi in range(2):
                for bo in range(2):
                    m2[bi * 64 : bi * 64 + 64, bo * 64 : bo * 64 + 64] = (
                        w2[2 * p + bo, 2 * q + bi] * i64
                    )
                    m3[bi * 64 : bi * 64 + 64, bo * 64 : bo * 64 + 64] = (
                        w3[2 * p + bo, 2 * q + bi] * i64
                    )
    wm[44] = np.eye(128, dtype=np.float32)

    bvv = np.zeros((128, 12), np.float32)
    for q in range(4):
        for b in range(2):
            sl = slice(b * 64, b * 64 + 64)
            bvv[sl, q] = bias1[2 * q + b]
            bvv[sl, 4 + q] = b2[2 * q + b]
            bvv[sl, 8 + q] = b3[2 * q + b]
    return wm.astype(F16), bvv


def _pack_psi(psi):
    """[BS,H,RES,RES] -> [NCORES, 4, 128, IPC*XP] fp16: partition p=(b,y) of
    channel pair q, free = (image, padded x); per-partition rows contiguous."""
    psip = np.zeros((BS, H, RES, XP), np.float32)
    psip[:, :, :, 2 : 2 + RES] = psi
    v = psip.reshape(NCORES, IPC, 4, 2, RES, XP)
    v = v.transpose(0, 2, 3, 4, 1, 5)  # [NCORES, 4, 2, RES, IPC, XP]
    return np.ascontiguousarray(v).reshape(NCORES, 4, 128, IPC * XP).astype(F16)


def _unpack_out(parts):
    """list of [4,128,IPC*RES] fp16 per core -> [BS,H,RES,RES] fp32"""
    v = np.stack([np.asarray(p).astype(np.float32) for p in parts])
    v = v.reshape(NCORES, 4, 2, RES, IPC, RES)
    v = v.transpose(0, 4, 1, 2, 3, 5)  # [NCORES, IPC, 4, 2, RES, RES]
    return np.ascontiguousarray(v).reshape(BS, H, RES, RES)


def kernel(psi, filter1, bias1, w2, b2, w3, b3):
    from concourse.bass_utils import run_bass_kernel_spmd

    psi = np.asarray(psi, dtype=np.float32)
    wm, bvv = _host_pack(
        np.asarray(filter1, np.float32),
        np.asarray(bias1, np.float32),
        np.asarray(w2, np.float32),
        np.asarray(b2, np.float32),
        np.asarray(w3, np.float32),
        np.asarray(b3, np.float32),
    )

    psit = _pack_psi(psi)

    if "nc" not in _CACHE:
        _CACHE["nc"] = _build_program()
    nc = _CACHE["nc"]

    in_maps = [{"psi": psit[c], "wm": wm, "bv": bvv} for c in range(NCORES)]
    res = run_bass_kernel_spmd(nc, in_maps, list(range(NCORES)))
    return _unpack_out([r["out"] for r in res.results])


# revision 7
# speedup vs baseline: 1.7964x; 1.3972x over previous
"""Trainium2 Bass kernel for the nn_CA depthwise-conv CA step.

Pipeline per image: depthwise 5x5 conv (D4-symmetrized, zero-mean kernel,
SAME padding) + bias + leaky_relu; 1x1 conv (8x8 channel GEMM) + bias +
leaky_relu; 1x1 conv + bias + psi residual + tanh.

Strategy: pure data parallel over 8 NeuronCores (256 images each).
On-chip layout: partition p = h*16 + ylocal, one tile per y-quarter
(y = qy*16 + ylocal) holding ALL 8 channels; free dim = (image, x).
All data is fp16.

With all 8 channels in the partition dim, each 8x8 channel GEMM is a
SINGLE matmul per quarter (lhsT = W[co,ci] (x) I16) — stages 2+3 are 8
matmuls per 8-image step vs 32 in a (2ch x 64y) pair layout. The
depthwise conv uses the D4 x-symmetry (kernel columns 0==4, 1==3): per
quarter it is 3 banded-Toeplitz matmuls (center column, s1 =
psi(x-1)+psi(x+1), s2 = psi(x-2)+psi(x+2); s tiles built on the idle
vector engine), plus corner-block "spill" matmuls carrying the
y-convolution across quarter boundaries (up to 6 per quarter). 38
matmuls per step total. Per-quarter PSUM tiles keep dependency tracking
fine-grained: conv accumulators use 2 banks double-buffered, leaving 6
banks for the GEMM/act/residual chain. The psi residual is added by the
vector engine in PSUM; biases + activations are fused into ScalarE
activation instructions reading PSUM.
"""

import numpy as np

F16 = np.dtype(np.float16)

BS, H, RES = 2048, 8, 64
NCORES = 8
IPC = BS // NCORES  # images per core
SG = 32             # images per super-group (DMA granularity)
NSG = IPC // SG
S8 = 8              # images per PSUM step (512 free columns per quarter)
NST = SG // S8
XP = RES + 4        # x-padded width
YQ = 16             # y rows per quarter tile

NWM = 12            # 3 main conv + 3 spill-up + 3 spill-down + gemm2 + gemm3 + I

_CACHE = {}


def _totalistic(x):
    # D4-symmetrize 5x5 kernels over spatial dims, then remove spatial mean
    z = 0.125 * (x + x[:, :, ::-1, :] + x[:, :, :, ::-1] + x[:, :, ::-1, ::-1])
    xt = np.swapaxes(x, 2, 3)
    z = z + 0.125 * (xt + xt[:, :, ::-1, :] + xt[:, :, :, ::-1] + xt[:, :, ::-1, ::-1])
    return z - z.mean(axis=(2, 3), keepdims=True)


def _build_program(reps=1, resid_on_pe=False):
    import concourse.bacc as bacc
    import concourse.tile as tile
    from concourse import mybir

    dt = mybir.dt
    nc = bacc.Bacc("TRN2", target_bir_lowering=False, debug=False, num_devices=NCORES)
    psi = nc.dram_tensor("psi", [4, 128, IPC * XP], dt.float16, kind="ExternalInput").ap()
    wm = nc.dram_tensor("wm", [NWM, 128, 128], dt.float16, kind="ExternalInput").ap()
    bv = nc.dram_tensor("bv", [128, 2], dt.float32, kind="ExternalInput").ap()
    out = nc.dram_tensor("out", [4, 128, IPC * RES], dt.float16, kind="ExternalOutput").ap()

    LR = mybir.ActivationFunctionType.Lrelu
    TH = mybir.ActivationFunctionType.Tanh

    with tile.TileContext(nc) as tc:
        from contextlib import ExitStack

        with ExitStack() as ctx:
            const = ctx.enter_context(tc.tile_pool(name="const", bufs=1))
            psip = ctx.enter_context(tc.tile_pool(name="psip", bufs=2))
            spool = ctx.enter_context(tc.tile_pool(name="spool", bufs=2))
            opool = ctx.enter_context(tc.tile_pool(name="opool", bufs=2))
            zpool = ctx.enter_context(tc.tile_pool(name="zpool", bufs=2))
            psum = ctx.enter_context(tc.tile_pool(name="psum", bufs=6, space="PSUM"))

            wt = const.tile([128, NWM * 128], dt.float16)
            nc.sync.dma_start(
                wt[:].rearrange("p (m k) -> p m k", m=NWM),
                wm.rearrange("m p k -> p m k"),
            )
            bt = const.tile([128, 2], dt.float32)
            nc.sync.dma_start(bt[:], bv[:])

            def W(i):
                return wt[:, i * 128 : (i + 1) * 128]

            for rep in range(reps):
              for sg in range(NSG):
                ptiles, s1tiles, s2tiles = [], [], []
                for v in range(4):
                    t = psip.tile([128, SG * XP], dt.float16, tag=f"psi{v}", name=f"psi_t{v}_{sg}")
                    nc.sync.dma_start(
                        t[:], psi[v, :, sg * SG * XP : (sg + 1) * SG * XP]
                    )
                    ptiles.append(t)
                    pv = t[:].rearrange("p (i x) -> p i x", i=SG)
                    t1 = spool.tile([128, SG * RES], dt.float16, tag=f"s1_{v}", name=f"s1_t{v}_{sg}")
                    t2 = spool.tile([128, SG * RES], dt.float16, tag=f"s2_{v}", name=f"s2_t{v}_{sg}")
                    v1 = t1[:].rearrange("p (i x) -> p i x", i=SG)
                    v2 = t2[:].rearrange("p (i x) -> p i x", i=SG)
                    nc.vector.tensor_add(v1, pv[:, :, 1 : 1 + RES], pv[:, :, 3 : 3 + RES])
                    nc.vector.tensor_add(v2, pv[:, :, 0:RES], pv[:, :, 4 : 4 + RES])
                    s1tiles.append(t1)
                    s2tiles.append(t2)
                otiles = [
                    opool.tile([128, SG * RES], dt.float16, tag=f"o{v}", name=f"o_t{v}_{sg}")
                    for v in range(4)
                ]

                def views(v, st):
                    # (center, s1, s2) rhs views of quarter v for step st
                    pt3 = ptiles[v][:].rearrange("p (i x) -> p i x", i=SG)[
                        :, st * S8 : (st + 1) * S8, :
                    ]
                    s13 = s1tiles[v][:].rearrange("p (i x) -> p i x", i=SG)[
                        :, st * S8 : (st + 1) * S8, :
                    ]
                    s23 = s2tiles[v][:].rearrange("p (i x) -> p i x", i=SG)[
                        :, st * S8 : (st + 1) * S8, :
                    ]
                    return (pt3[:, :, 2 : 2 + RES], s13, s23)

                for st in range(NST):
                    # stage 1: depthwise conv + lrelu (bias1 is always 0).
                    # One 1-bank accumulator per quarter, double-buffered.
                    z1 = []
                    for v in range(4):
                        cps = psum.tile(
                            [128, S8 * RES], dt.float32,
                            tag="cps", name=f"cps{sg}_{st}_{v}", bufs=2,
                        )
                        cps3 = cps[:].rearrange("p (i x) -> p i x", i=S8)
                        mm = []
                        own = views(v, st)
                        for c in range(3):  # lhsT c: x-class (0=s2,1=s1,2=center)
                            mm.append((W(c), own[2 - c]))
                        if v > 0:
                            below = views(v - 1, st)
                            for c in range(3):
                                mm.append((W(3 + c), below[2 - c]))
                        if v < 3:
                            above = views(v + 1, st)
                            for c in range(3):
                                mm.append((W(6 + c), above[2 - c]))
                        for j, (lhs, rhs) in enumerate(mm):
                            nc.tensor.matmul(
                                cps3, lhs, rhs,
                                start=(j == 0), stop=(j == len(mm) - 1),
                            )
                        z = zpool.tile(
                            [128, S8 * RES], dt.float16,
                            tag=f"z1_{v}", name=f"z1_{sg}_{st}_{v}",
                        )
                        nc.scalar.activation(z[:], cps[:], LR, alpha=0.01)
                        z1.append(z)
                    # stage 2: 8x8 channel GEMM + b2 + lrelu (one matmul/quarter)
                    z2 = []
                    for v in range(4):
                        gps = psum.tile(
                            [128, S8 * RES], dt.float32, tag="ps", name=f"g2ps{sg}_{st}_{v}",
                        )
                        nc.tensor.matmul(
                            gps[:], W(9), z1[v][:], start=True, stop=True,
                        )
                        z = zpool.tile([128, S8 * RES], dt.float16, tag=f"z2_{v}", name=f"z2_{sg}_{st}_{v}")
                        nc.scalar.activation(
                            z[:], gps[:], LR, bias=bt[:, 0:1], alpha=0.01
                        )
                        z2.append(z)
                    # stage 3: 8x8 channel GEMM + psi residual + b3 + tanh
                    for v in range(4):
                        gps = psum.tile(
                            [128, S8 * RES], dt.float32, tag="ps", name=f"g3ps{sg}_{st}_{v}",
                        )
                        pt3 = ptiles[v][:].rearrange("p (i x) -> p i x", i=SG)[
                            :, st * S8 : (st + 1) * S8, 2 : 2 + RES
                        ]
                        gps3 = gps[:].rearrange("p (i x) -> p i x", i=S8)
                        if resid_on_pe:
                            nc.tensor.matmul(gps3, W(11), pt3, start=True, stop=False)
                        nc.tensor.matmul(
                            gps[:], W(10), z2[v][:],
                            start=not resid_on_pe, stop=True,
                        )
                        if not resid_on_pe:
                            # psi residual on the vector engine
                            nc.vector.tensor_add(gps3, gps3, pt3)
                        nc.scalar.activation(
                            otiles[v][:, st * S8 * RES : (st + 1) * S8 * RES],
                            gps[:], TH, bias=bt[:, 1:2],
                        )
                for v in range(4):
                    nc.sync.dma_start(
                        out[v, :, sg * SG * RES : (sg + 1) * SG * RES],
                        otiles[v][:],
                    )

    nc.compile()
    return nc


def _host_pack(filter1, bias1, w2, b2, w3, b3):
    w = _totalistic(filter1.astype(np.float32))[:, 0]  # [8,5,5]
    wm = np.zeros((NWM, 128, 128), np.float32)
    # main banded conv matrices (same local structure for every quarter):
    # lhsT[ch*16+yi, ch*16+yo] = K[ch, (yi-yo)+2, c]
    for c in range(3):
        for ch in range(H):
            o = ch * YQ
            for yi in range(YQ):
                for yo in range(YQ):
                    d = yi - yo
                    if -2 <= d <= 2:
                        wm[c, o + yi, o + yo] = w[ch, d + 2, c]
            # spill up: src quarter u -> dst u+1; dy = ys - 16 - yd in {-2,-1}
            for ys, yd in ((14, 0), (15, 0), (15, 1)):
                wm[3 + c, o + ys, o + yd] = w[ch, (ys - 16 - yd) + 2, c]
            # spill down: src quarter u -> dst u-1; dy = ys + 16 - yd in {1,2}
            for ys, yd in ((0, 14), (0, 15), (1, 15)):
                wm[6 + c, o + ys, o + yd] = w[ch, (ys + 16 - yd) + 2, c]
    # channel GEMMs: lhsT[ci*16+y, co*16+y] = W[co, ci]
    i16 = np.eye(YQ, dtype=np.float32)
    for co in range(H):
        for ci in range(H):
            wm[9, ci * YQ : ci * YQ + YQ, co * YQ : co * YQ + YQ] = w2[co, ci] * i16
            wm[10, ci * YQ : ci * YQ + YQ, co * YQ : co * YQ + YQ] = w3[co, ci] * i16
    wm[11] = np.eye(128, dtype=np.float32)

    bvv = np.zeros((128, 2), np.float32)
    for ch in range(H):
        sl = slice(ch * YQ, ch * YQ + YQ)
        bvv[sl, 0] = b2[ch]
        bvv[sl, 1] = b3[ch]
    return wm.astype(F16), bvv


def _pack_psi(psi):
    """[BS,H,RES,RES] -> [NCORES, 4, 128, IPC*XP] fp16: tile axis = y-quarter,
    partition p = ch*16 + ylocal, free = (image, padded x)."""
    psip = np.zeros((BS, H, RES, XP), np.float32)
    psip[:, :, :, 2 : 2 + RES] = psi
    v = psip.reshape(NCORES, IPC, H, 4, YQ, XP)
    v = v.transpose(0, 3, 2, 4, 1, 5)  # [NCORES, qy, ch, ylocal, IPC, XP]
    return np.ascontiguousarray(v).reshape(NCORES, 4, 128, IPC * XP).astype(F16)


def _unpack_out(parts):
    """list of [4,128,IPC*RES] fp16 per core -> [BS,H,RES,RES] fp32"""
    v = np.stack([np.asarray(p).astype(np.float32) for p in parts])
    v = v.reshape(NCORES, 4, H, YQ, IPC, RES)
    v = v.transpose(0, 4, 2, 1, 3, 5)  # [NCORES, IPC, ch, qy, ylocal, RES]
    return np.ascontiguousarray(v).reshape(BS, H, RES, RES)


def kernel(psi, filter1, bias1, w2, b2, w3, b3):
    from concourse.bass_utils import run_bass_kernel_spmd

    psi = np.asarray(psi, dtype=np.float32)
    wm, bvv = _host_pack(
        np.asarray(filter1, np.float32),
        np.asarray(bias1, np.float32),
        np.asarray(w2, np.float32),
        np.asarray(b2, np.float32),
        np.asarray(w3, np.float32),
        np.asarray(b3, np.float32),
    )

    psit = _pack_psi(psi)

    if "nc" not in _CACHE:
        _CACHE["nc"] = _build_program()
    nc = _CACHE["nc"]

    in_maps = [{"psi": psit[c], "wm": wm, "bv": bvv} for c in range(NCORES)]
    res = run_bass_kernel_spmd(nc, in_maps, list(range(NCORES)))
    return _unpack_out([r["out"] for r in res.results])
